# revision 21
# baseline (speedup 1.0000x reference)
"""Trainium2 Bass kernel for AntecedentShareGMF (fuzzy rule softmax).

Math: X [N, D], center/sigma [D, M], M=2, R = M^D = 1024 rules; rule r picks
MF index i(r,d) = bit (D-1-d) of r:
    z[n, r] = (1/D) * sum_d -0.5 * (X[n,d] - C[r,d])^2 / (S[r,d]^2 + eps)
    out = softmax_r(z)

Since B[d,r] = i(r,d) is 0/1, every per-rule coefficient is AFFINE in B:
    w    = w0 + (w1-w0) B          (w_m = -0.5/D/(sigma_m^2+eps))
    -2wC = a0 + (a1-a0) B          (a_m = -2 w_m c_m)
    wC^2 = g0 + (g1-g0) B          (g_m = w_m c_m^2)
so z[n,r] is ONE K=128 matmul per 128-sample tile against a weight tensor
whose row blocks sit at 32-aligned partition offsets (so each runtime
transform is a legal in-place engine op — no partition-assembly DMAs):
    rhs  Wp rows:  0..9 a-rows | 32..41 w-rows | 64..73 gdiff*B |
                   96..105 g0*ones | rest zero
    lhsT xt rows:  0..9 x | 32..41 x^2 | 64..73 + 96..105 ones | rest garbage
The B/ones/zeros table is input-independent -> baked into the NEFF via
inline_tensor; only 4 aligned [10, R] transforms depend on center/sigma.
Matmuls run as float32r (full-rate f32 streaming, ~22-bit mantissa).
Softmax: z in [-3.3, 0) for this distribution -> no max subtraction needed;
exp+row-sum fused in one ScalarE activation, divide on VectorE.

Data-parallel over N across 8 cores; no cross-core communication.
"""

import numpy as np

import concourse.bass as bass
import concourse.bacc as bacc
import concourse.tile as tile
from concourse import mybir
from concourse.bass_utils import run_bass_kernel_spmd

N, D, M = 8192, 10, 2
R = M**D  # 1024
NCORES = 8
NSHARD = N // NCORES  # 1024
P = 128
NTILES = NSHARD // P  # 8
EPS = 1e-8
F32 = mybir.dt.float32
F32R = mybir.dt.float32r
HR = 512  # half of R; one PSUM bank / max f32 matmul free size
AF = mybir.ActivationFunctionType
ALU = mybir.AluOpType


def _bit_table() -> np.ndarray:
    r = np.arange(R, dtype=np.int64)
    return np.stack(
        [((r >> (D - 1 - d)) & 1).astype(np.float32) for d in range(D)]
    )  # [D, R]


def build_nc() -> bass.Bass:
    nc = bacc.Bacc()
    X = nc.declare_dram_parameter("X", [NSHARD, D], F32, isOutput=False)
    center = nc.declare_dram_parameter("center", [D, M], F32, isOutput=False)
    sigma = nc.declare_dram_parameter("sigma", [D, M], F32, isOutput=False)
    out = nc.declare_dram_parameter("out", [NSHARD, R], F32, isOutput=True)

    B = _bit_table()
    bwc = np.zeros((P, R), np.float32)
    bwc[0:D] = B  # -> a-rows
    bwc[32 : 32 + D] = B  # -> w-rows
    bwc[64 : 64 + D] = B  # -> gdiff*B rows
    bwc[96 : 96 + D] = 1.0  # -> g0 rows
    bwc_d = nc.inline_tensor(bwc, name="bwc")
    ident_d = nc.inline_tensor(np.eye(P, dtype=np.float32), name="ident")

    with tile.TileContext(nc) as tc:
        with (
            tc.tile_pool(name="consts", bufs=1) as consts,
            tc.tile_pool(name="xt", bufs=4) as xt_pool,
            tc.tile_pool(name="prob", bufs=2) as prob_pool,
            tc.tile_pool(name="stat", bufs=8) as stat_pool,
            tc.tile_pool(name="pt", bufs=2, space="PSUM") as pt_pool,
            tc.tile_pool(name="pz", bufs=3, space="PSUM") as pz_pool,
        ):
            # input param loads first (tiny; on the setup critical path)
            cen = consts.tile([D, M], F32)
            sig = consts.tile([D, M], F32)
            nc.sync.dma_start(out=cen, in_=center[:, :])
            nc.sync.dma_start(out=sig, in_=sigma[:, :])

            # persistent X staging tiles: ones/zero columns written once, only
            # X and X^2 columns change per use
            xes = [
                consts.tile([P, P], F32, name=f"xe{i}", tag=f"xe{i}")
                for i in range(4)
            ]
            for xe in xes:
                nc.vector.memset(
                    xe.rearrange("p (q c) -> p q c", c=32)[:, 2:4, 0:D], 1.0
                )
                nc.vector.memset(
                    xe.rearrange("p (q c) -> p q c", c=32)[:, :, D:32], 0.0
                )
            for t in range(4):
                nc.sync.dma_start(
                    out=xes[t % 4][:, 0:D], in_=X[t * P : (t + 1) * P, :]
                )

            ident = consts.tile([P, P], F32)
            nc.sync.dma_start(out=ident, in_=ident_d[:, :])
            Wp = consts.tile([P, R], F32)
            nc.sync.dma_start(out=Wp, in_=bwc_d[:, :])

            # tiny [D, M] prep: w, a, g coefficient pairs
            epsb = consts.tile([D, 1], F32)
            nc.vector.memset(epsb, EPS)
            sq = consts.tile([D, M], F32)
            nc.vector.tensor_mul(out=sq, in0=sig, in1=sig)
            nc.vector.tensor_scalar_add(out=sq, in0=sq, scalar1=epsb)
            w01 = consts.tile([D, M], F32)
            nc.vector.reciprocal(out=w01, in_=sq)
            nc.vector.tensor_scalar_mul(out=w01, in0=w01, scalar1=-0.5 / D)
            wc01 = consts.tile([D, M], F32)
            nc.vector.tensor_mul(out=wc01, in0=w01, in1=cen)  # w*c
            a01 = consts.tile([D, M], F32)
            nc.vector.tensor_scalar_mul(out=a01, in0=wc01, scalar1=-2.0)
            g01 = consts.tile([D, M], F32)
            nc.vector.tensor_mul(out=g01, in0=wc01, in1=cen)  # w*c^2
            adiff = consts.tile([D, 1], F32)
            nc.vector.tensor_sub(out=adiff, in0=a01[:, 1:2], in1=a01[:, 0:1])
            wdiff = consts.tile([D, 1], F32)
            nc.vector.tensor_sub(out=wdiff, in0=w01[:, 1:2], in1=w01[:, 0:1])
            gdiff = consts.tile([D, 1], F32)
            nc.vector.tensor_sub(out=gdiff, in0=g01[:, 1:2], in1=g01[:, 0:1])

            # in-place W transforms at 32-aligned partition offsets; outputs
            # f32r-rounded (consumed by fp32r matmuls). Split ACT/DVE.
            nc.scalar.activation(
                out=Wp[0:D, :].bitcast(F32R), in_=Wp[0:D, :], func=AF.Identity,
                bias=a01[:, 0:1], scale=adiff,
            )
            nc.vector.tensor_scalar(
                out=Wp[32 : 32 + D, :].bitcast(F32R), in0=Wp[32 : 32 + D, :],
                scalar1=wdiff, scalar2=w01[:, 0:1], op0=ALU.mult, op1=ALU.add,
            )
            nc.scalar.activation(
                out=Wp[64 : 64 + D, :].bitcast(F32R), in_=Wp[64 : 64 + D, :],
                func=AF.Identity, bias=0.0, scale=gdiff,
            )
            nc.vector.tensor_scalar_mul(
                out=Wp[96 : 96 + D, :].bitcast(F32R), in0=Wp[96 : 96 + D, :],
                scalar1=g01[:, 0:1],
            )

            for t in range(NTILES):
                xe = xes[t % 4]
                nc.scalar.activation(
                    out=xe[:, 32 : 32 + D], in_=xe[:, 0:D], func=AF.Square
                )

                pt = pt_pool.tile([P, P], F32)
                nc.tensor.transpose(out=pt, in_=xe, identity=ident)
                if t + 4 < NTILES:
                    # refill this staging buffer for tile t+4
                    nc.sync.dma_start(
                        out=xe[:, 0:D], in_=X[(t + 4) * P : (t + 5) * P, :]
                    )
                xt = xt_pool.tile([P, P], F32)
                nc.vector.tensor_copy(out=xt.bitcast(F32R), in_=pt)

                if t % 2 == 0:
                    prob = prob_pool.tile([P, 2, R], F32)
                pz = pz_pool.tile([P, R], F32)
                for h in range(2):
                    nc.tensor.matmul(
                        out=pz[:, h * HR : (h + 1) * HR],
                        lhsT=xt[:, :].bitcast(F32R),
                        rhs=Wp[:, h * HR : (h + 1) * HR].bitcast(F32R),
                    )
                sums = stat_pool.tile([P, 1], F32)
                nc.scalar.activation(
                    out=prob[:, t % 2, :], in_=pz, func=AF.Exp, bias=0.0,
                    scale=1.0, accum_out=sums,
                )
                rsum = stat_pool.tile([P, 1], F32)
                nc.vector.reciprocal(out=rsum, in_=sums)
                nc.vector.tensor_scalar_mul(
                    out=prob[:, t % 2, :], in0=prob[:, t % 2, :], scalar1=rsum
                )
                if t % 2 == 1:
                    # one 1MB DMA per tile pair
                    nc.sync.dma_start(
                        out=out[(t - 1) * P : (t + 1) * P, :].rearrange(
                            "(b p) r -> p b r", p=P
                        ),
                        in_=prob,
                    )

    return nc


_NC_CACHE: list = []


def _get_nc() -> bass.Bass:
    if not _NC_CACHE:
        nc = build_nc()
        if not nc.is_finalized():
            nc.finalize()  # runs Bacc.compile (wait splitting, reg alloc)
        _NC_CACHE.append(nc)
    return _NC_CACHE[0]


def run(X, center, sigma, **spmd_kwargs):
    X = np.ascontiguousarray(np.asarray(X, dtype=np.float32))
    center = np.ascontiguousarray(np.asarray(center, dtype=np.float32))
    sigma = np.ascontiguousarray(np.asarray(sigma, dtype=np.float32))
    nc = _get_nc()
    in_maps = [
        {"X": X[i * NSHARD : (i + 1) * NSHARD], "center": center, "sigma": sigma}
        for i in range(NCORES)
    ]
    res = run_bass_kernel_spmd(nc, in_maps, core_ids=list(range(NCORES)), **spmd_kwargs)
    out = np.concatenate(
        [np.asarray(res.results[i]["out"]) for i in range(NCORES)], axis=0
    )
    return out, res


def kernel(**inputs) -> np.ndarray:
    out, _ = run(inputs["X"], inputs["center"], inputs["sigma"])
    return out


# revision 23
# speedup vs baseline: 1.2208x; 1.2208x over previous
"""Trainium2 Bass kernel for AntecedentShareGMF (fuzzy rule softmax).

Math: X [N, D], center/sigma [D, M], M=2, R = M^D = 1024 rules; rule r picks
MF index i(r,d) = bit (D-1-d) of r:
    z[n, r] = (1/D) * sum_d -0.5 * (X[n,d] - C[r,d])^2 / (S[r,d]^2 + eps)
    out = softmax_r(z)

Since B[d,r] = i(r,d) is 0/1, every per-rule coefficient is AFFINE in B:
    w    = w0 + (w1-w0) B          (w_m = -0.5/D/(sigma_m^2+eps))
    -2wC = a0 + (a1-a0) B          (a_m = -2 w_m c_m)
    wC^2 = g0 + (g1-g0) B          (g_m = w_m c_m^2)
so z[n,r] is ONE K=128 matmul per 128-sample tile against a weight tensor
whose row blocks sit at 32-aligned partition offsets (so each runtime
transform is a legal in-place engine op — no partition-assembly DMAs):
    rhs  Wp rows:  0..9 a-rows | 32..41 w-rows | 64..73 gdiff*B |
                   96..105 g0*ones | rest zero
    lhsT xt rows:  0..9 x | 32..41 x^2 | 64..73 + 96..105 ones | rest garbage
The B/ones/zeros table is input-independent -> baked into the NEFF via
inline_tensor; only 4 aligned [10, R] transforms depend on center/sigma.
Matmuls run as float32r (full-rate f32 streaming, ~22-bit mantissa).
Softmax: z in [-3.3, 0) for this distribution -> no max subtraction needed;
exp+row-sum fused in one ScalarE activation, divide on VectorE.

Data-parallel over N across 8 cores; no cross-core communication.
"""

import numpy as np

import concourse.bass as bass
import concourse.bacc as bacc
import concourse.tile as tile
from concourse import mybir
from concourse.bass_utils import run_bass_kernel_spmd

N, D, M = 8192, 10, 2
R = M**D  # 1024
NCORES = 8
NSHARD = N // NCORES  # 1024
P = 128
NTILES = NSHARD // P  # 8
EPS = 1e-8
F32 = mybir.dt.float32
F32R = mybir.dt.float32r
HR = 512  # half of R; one PSUM bank / max f32 matmul free size
AF = mybir.ActivationFunctionType
ALU = mybir.AluOpType


def _bit_table() -> np.ndarray:
    r = np.arange(R, dtype=np.int64)
    return np.stack(
        [((r >> (D - 1 - d)) & 1).astype(np.float32) for d in range(D)]
    )  # [D, R]


def build_nc() -> bass.Bass:
    nc = bacc.Bacc()
    X = nc.declare_dram_parameter("X", [NSHARD, D], F32, isOutput=False)
    center = nc.declare_dram_parameter("center", [D, M], F32, isOutput=False)
    sigma = nc.declare_dram_parameter("sigma", [D, M], F32, isOutput=False)
    out = nc.declare_dram_parameter("out", [NSHARD, R], F32, isOutput=True)

    B = _bit_table()
    bwc = np.concatenate([B, B, B, np.ones((D, R), np.float32)])  # [40, R]
    bwc_d = nc.inline_tensor(bwc, name="bwc")
    ident_d = nc.inline_tensor(np.eye(P, dtype=np.float32), name="ident")

    with tile.TileContext(nc) as tc:
        with (
            tc.tile_pool(name="consts", bufs=1) as consts,
            tc.tile_pool(name="xt", bufs=4) as xt_pool,
            tc.tile_pool(name="prob", bufs=4) as prob_pool,
            tc.tile_pool(name="stat", bufs=8) as stat_pool,
            tc.tile_pool(name="pt", bufs=2, space="PSUM") as pt_pool,
            tc.tile_pool(name="pz", bufs=3, space="PSUM") as pz_pool,
        ):
            # input param loads first (tiny; on the setup critical path);
            # DMAs spread across engine DGE paths to avoid FIFO stacking
            cen = consts.tile([D, M], F32)
            sig = consts.tile([D, M], F32)
            nc.gpsimd.dma_start(out=cen, in_=center[:, :])
            nc.gpsimd.dma_start(out=sig, in_=sigma[:, :])

            # Wp: zero fill + 4 aligned 40KB block loads on 4 separate paths
            Wp = consts.tile([P, R], F32)
            nc.vector.memset(Wp, 0.0)
            for i, (eng, row) in enumerate(
                zip((nc.sync, nc.scalar, nc.sync, nc.scalar), (0, 32, 64, 96))
            ):
                eng.dma_start(
                    out=Wp[row : row + D, :], in_=bwc_d[i * D : (i + 1) * D, :]
                )

            # persistent X staging tiles: ones/zero columns written once, only
            # X and X^2 columns change per use
            xes = [
                consts.tile([P, P], F32, name=f"xe{i}", tag=f"xe{i}")
                for i in range(4)
            ]
            for xe in xes:
                nc.vector.memset(
                    xe.rearrange("p (q c) -> p q c", c=32)[:, 2:4, 0:D], 1.0
                )
                nc.vector.memset(
                    xe.rearrange("p (q c) -> p q c", c=32)[:, :, D:32], 0.0
                )
            for t in range(4):
                nc.gpsimd.dma_start(
                    out=xes[t % 4][:, 0:D], in_=X[t * P : (t + 1) * P, :]
                )

            ident = consts.tile([P, P], F32)
            nc.scalar.dma_start(out=ident, in_=ident_d[:, :])

            # tiny [D, M] prep: w, a, g coefficient pairs
            epsb = consts.tile([D, 1], F32)
            nc.vector.memset(epsb, EPS)
            sq = consts.tile([D, M], F32)
            nc.vector.tensor_mul(out=sq, in0=sig, in1=sig)
            nc.vector.tensor_scalar_add(out=sq, in0=sq, scalar1=epsb)
            w01 = consts.tile([D, M], F32)
            nc.vector.reciprocal(out=w01, in_=sq)
            nc.vector.tensor_scalar_mul(out=w01, in0=w01, scalar1=-0.5 / D)
            wc01 = consts.tile([D, M], F32)
            nc.vector.tensor_mul(out=wc01, in0=w01, in1=cen)  # w*c
            a01 = consts.tile([D, M], F32)
            nc.vector.tensor_scalar_mul(out=a01, in0=wc01, scalar1=-2.0)
            g01 = consts.tile([D, M], F32)
            nc.vector.tensor_mul(out=g01, in0=wc01, in1=cen)  # w*c^2
            adiff = consts.tile([D, 1], F32)
            nc.vector.tensor_sub(out=adiff, in0=a01[:, 1:2], in1=a01[:, 0:1])
            wdiff = consts.tile([D, 1], F32)
            nc.vector.tensor_sub(out=wdiff, in0=w01[:, 1:2], in1=w01[:, 0:1])
            gdiff = consts.tile([D, 1], F32)
            nc.vector.tensor_sub(out=gdiff, in0=g01[:, 1:2], in1=g01[:, 0:1])

            # in-place W transforms at 32-aligned partition offsets; outputs
            # f32r-rounded (consumed by fp32r matmuls). Split ACT/DVE.
            nc.scalar.activation(
                out=Wp[0:D, :].bitcast(F32R), in_=Wp[0:D, :], func=AF.Identity,
                bias=a01[:, 0:1], scale=adiff,
            )
            nc.vector.tensor_scalar(
                out=Wp[32 : 32 + D, :].bitcast(F32R), in0=Wp[32 : 32 + D, :],
                scalar1=wdiff, scalar2=w01[:, 0:1], op0=ALU.mult, op1=ALU.add,
            )
            nc.scalar.activation(
                out=Wp[64 : 64 + D, :].bitcast(F32R), in_=Wp[64 : 64 + D, :],
                func=AF.Identity, bias=0.0, scale=gdiff,
            )
            nc.vector.tensor_scalar_mul(
                out=Wp[96 : 96 + D, :].bitcast(F32R), in0=Wp[96 : 96 + D, :],
                scalar1=g01[:, 0:1],
            )

            for t in range(NTILES):
                xe = xes[t % 4]
                nc.vector.tensor_mul(
                    out=xe[:, 32 : 32 + D], in0=xe[:, 0:D], in1=xe[:, 0:D]
                )

                pt = pt_pool.tile([P, P], F32)
                nc.tensor.transpose(out=pt, in_=xe, identity=ident)
                if t + 4 < NTILES:
                    # refill this staging buffer for tile t+4
                    nc.gpsimd.dma_start(
                        out=xe[:, 0:D], in_=X[(t + 4) * P : (t + 5) * P, :]
                    )
                xt = xt_pool.tile([P, P], F32)
                nc.vector.tensor_copy(out=xt.bitcast(F32R), in_=pt)

                if t % 2 == 0:
                    prob = prob_pool.tile([P, 2, R], F32)
                pz = pz_pool.tile([P, R], F32)
                for h in range(2):
                    nc.tensor.matmul(
                        out=pz[:, h * HR : (h + 1) * HR],
                        lhsT=xt[:, :].bitcast(F32R),
                        rhs=Wp[:, h * HR : (h + 1) * HR].bitcast(F32R),
                    )
                sums = stat_pool.tile([P, 1], F32)
                nc.scalar.activation(
                    out=prob[:, t % 2, :], in_=pz, func=AF.Exp, bias=0.0,
                    scale=1.0, accum_out=sums,
                )
                rsum = stat_pool.tile([P, 1], F32)
                nc.vector.reciprocal(out=rsum, in_=sums)
                nc.vector.tensor_scalar_mul(
                    out=prob[:, t % 2, :], in0=prob[:, t % 2, :], scalar1=rsum
                )
                if t % 2 == 1:
                    # one 1MB DMA per tile pair, alternating DGE paths
                    (nc.sync if t % 4 == 1 else nc.scalar).dma_start(
                        out=out[(t - 1) * P : (t + 1) * P, :].rearrange(
                            "(b p) r -> p b r", p=P
                        ),
                        in_=prob,
                    )

    return nc


_NC_CACHE: list = []


def _get_nc() -> bass.Bass:
    if not _NC_CACHE:
        nc = build_nc()
        if not nc.is_finalized():
            nc.finalize()  # runs Bacc.compile (wait splitting, reg alloc)
        _NC_CACHE.append(nc)
    return _NC_CACHE[0]


def run(X, center, sigma, **spmd_kwargs):
    X = np.ascontiguousarray(np.asarray(X, dtype=np.float32))
    center = np.ascontiguousarray(np.asarray(center, dtype=np.float32))
    sigma = np.ascontiguousarray(np.asarray(sigma, dtype=np.float32))
    nc = _get_nc()
    in_maps = [
        {"X": X[i * NSHARD : (i + 1) * NSHARD], "center": center, "sigma": sigma}
        for i in range(NCORES)
    ]
    res = run_bass_kernel_spmd(nc, in_maps, core_ids=list(range(NCORES)), **spmd_kwargs)
    out = np.concatenate(
        [np.asarray(res.results[i]["out"]) for i in range(NCORES)], axis=0
    )
    return out, res


def kernel(**inputs) -> np.ndarray:
    out, _ = run(inputs["X"], inputs["center"], inputs["sigma"])
    return out
